# revision 11
# baseline (speedup 1.0000x reference)
"""Masked multi-head self-attention on 8 Trainium2 NeuronCores.

Sharding: core c handles batch b = c // 2 and head-group g = c % 2
(8 of 16 heads).  Data-parallel over B, tensor-parallel over heads for
qkv_proj (column split) / out_proj (row split).  The [T,T] causal mask
is exploited structurally (tile skipping); the host verifies the mask
is causal and falls back to numpy otherwise.  Host sums the two
head-group partial outputs per batch and adds bout.
"""

import numpy as np
import ml_dtypes

BF16 = ml_dtypes.bfloat16

B = 4
T = 2048
D = 1024
H = 16
DK = 64
P = 128
NCORES = 8

KT = D // P            # 8   k-tiles over d_model
TTILES = T // P        # 16  tiles over tokens
NCH = 4                # qi chunks of 512
CH = T // NCH          # 512
KITILES = T // P       # 16  ki tiles

_CACHE = {}


def _build_program():
    import concourse.bass as bass
    import concourse.tile as tile
    from concourse import bacc, mybir
    from contextlib import ExitStack

    f32 = mybir.dt.float32
    bf16 = mybir.dt.bfloat16
    nc = bacc.Bacc("TRN2", target_bir_lowering=False, debug=False,
                   num_devices=NCORES)

    xt_d = nc.declare_dram_parameter("xt", [P, KT * T], bf16, isOutput=False)
    wqk_d = nc.declare_dram_parameter("wqk", [P, 8 * 1024], bf16, isOutput=False)
    wv_d = nc.declare_dram_parameter("wv", [P, KT * 512], bf16, isOutput=False)
    wout_d = nc.declare_dram_parameter("wout", [P, 4 * 1024], bf16, isOutput=False)
    m01_d = nc.declare_dram_parameter("m01", [P, P], bf16, isOutput=False)
    bqk_d = nc.declare_dram_parameter("bqk", [P, 8], f32, isOutput=False)
    bv_d = nc.declare_dram_parameter("bv", [1, 512], bf16, isOutput=False)
    out_d = nc.declare_dram_parameter("out", [T, D], f32, isOutput=True)

    ts = bass.ts

    with tile.TileContext(nc) as tc, ExitStack() as top:
        const = top.enter_context(tc.tile_pool(name="const", bufs=1))
        qk_pool = top.enter_context(tc.tile_pool(name="qk", bufs=1))
        v_pool = top.enter_context(tc.tile_pool(name="v", bufs=1))

        ones_col = const.tile([P, 1], bf16, tag="ones_col")
        ones_row = const.tile([1, P], bf16, tag="ones_row")
        neg12 = const.tile([P, 1], f32, tag="neg12")
        bqk_sb = const.tile([P, 8], f32, tag="bqk")
        bv_sb = const.tile([1, 512], bf16, tag="bv")
        nc.vector.memset(ones_col[:], 1.0)
        nc.vector.memset(ones_row[:], 1.0)
        nc.vector.memset(neg12[:], -12.0)
        nc.sync.dma_start(bqk_sb[:], bqk_d[:])
        nc.sync.dma_start(bv_sb[:], bv_d[:])

        # Persistent intermediates: qkT [1024, T] as 8 tiles (i<4: q of head
        # pair i, scaled by 1/8 on host; i>=4: k of pair i-4); v [T, 512] as
        # 16 tiles; attnT [512, T] as 4 tiles.
        qk = [qk_pool.tile([P, T], bf16, tag=f"qk{i}", name=f"qk{i}")
              for i in range(8)]
        v = [v_pool.tile([P, 512], bf16, tag=f"v{t}", name=f"v{t}")
             for t in range(TTILES)]

        # ---- Phase 1: QKV projection -------------------------------------
        with ExitStack() as ph1:
            xt_pool = ph1.enter_context(tc.tile_pool(name="xt", bufs=1))
            wqk_pool = ph1.enter_context(tc.tile_pool(name="wqk", bufs=2))
            wv_pool = ph1.enter_context(tc.tile_pool(name="wv", bufs=1))
            ps1 = ph1.enter_context(tc.tile_pool(name="ps1", bufs=3, space="PSUM"))

            xt = []
            for kt in range(KT):
                xti = xt_pool.tile([P, T], bf16, tag=f"xt{kt}", name=f"xt{kt}")
                nc.sync.dma_start(xti[:], xt_d[:, ts(kt, T)])
                xt.append(xti)
            wv_sb = wv_pool.tile([P, KT * 512], bf16, tag="wv")
            nc.sync.dma_start(wv_sb[:], wv_d[:])

            # qkT[i] = (Wqk col-tile i).T @ x.T   -> [128 dcol, T]
            for i in range(8):
                wqk_sb = wqk_pool.tile([P, 1024], bf16, tag="wqk")
                nc.sync.dma_start(wqk_sb[:], wqk_d[:, ts(i, 1024)])
                for n in range(NCH):
                    acc = ps1.tile([P, CH], f32, tag="ps1")
                    for kt in range(KT):
                        nc.tensor.matmul(
                            acc[:], wqk_sb[:, ts(kt, P)], xt[kt][:, ts(n, CH)],
                            start=(kt == 0), stop=(kt == KT - 1))
                    nc.vector.tensor_scalar_add(
                        qk[i][:, ts(n, CH)], acc[:], bqk_sb[:, i:i + 1])

            # v[t] = x-tile.T @ Wv + bv           -> [128 tok, 512 dcol]
            for t in range(TTILES):
                acc = ps1.tile([P, 512], f32, tag="ps1")
                for kt in range(KT):
                    nc.tensor.matmul(
                        acc[:], xt[kt][:, ts(t, P)], wv_sb[:, ts(kt, 512)],
                        start=(kt == 0), stop=False)
                nc.tensor.matmul(acc[:], ones_row[:], bv_sb[:],
                                 start=False, stop=True)
                nc.vector.tensor_copy(v[t][:], acc[:])

        # ---- Phase 2: attention ------------------------------------------
        with ExitStack() as ph2:
            at_pool = ph2.enter_context(tc.tile_pool(name="at", bufs=1))
            wout_pool = ph2.enter_context(tc.tile_pool(name="wout", bufs=1))
            m01_pool = ph2.enter_context(tc.tile_pool(name="m01", bufs=1))
            pt_pool = ph2.enter_context(tc.tile_pool(name="pt", bufs=6))
            bc_pool = ph2.enter_context(tc.tile_pool(name="bc", bufs=2))
            scr_pool = ph2.enter_context(
                tc.tile_pool(name="scr", bufs=2, space="DRAM"))
            rs_pool = ph2.enter_context(tc.tile_pool(name="rs", bufs=2))
            at = [at_pool.tile([P, T], bf16, tag=f"at{p}", name=f"at{p}")
                  for p in range(4)]
            wout_sb = wout_pool.tile([P, 4 * 1024], bf16, tag="wout")
            nc.sync.dma_start(wout_sb[:], wout_d[:])
            m01_blk = m01_pool.tile([P, P], bf16, tag="m01")
            nc.sync.dma_start(m01_blk[:], m01_d[:])

            osb_pool = ph2.enter_context(tc.tile_pool(name="osb", bufs=4))
            with ExitStack() as ph2p:
                ps_s = ph2p.enter_context(
                    tc.tile_pool(name="ps_s", bufs=4, space="PSUM"))
                ps_at = ph2p.enter_context(
                    tc.tile_pool(name="ps_at", bufs=2, space="PSUM"))
                ps_rs = ph2p.enter_context(
                    tc.tile_pool(name="ps_rs", bufs=2, space="PSUM"))

                for c in range(NCH):
                    nki = 4 * (c + 1)  # causal: ki tiles 0..nki-1
                    for p in range(4):
                        kq = qk[4 + p]  # kT tiles for this pair
                        qq = qk[p]      # qT tiles
                        attn_ps = ps_at.tile([P, CH], f32, tag="at")
                        rs_ps = ps_rs.tile([33, CH], f32, tag="rs")
                        nc.vector.memset(rs_ps[0:32, :], 1.0)
                        for j in range(nki):
                            st = (j == 0)
                            sp = (j == nki - 1)
                            for h2 in range(2):  # head within pair
                                hs = slice(DK * h2, DK * (h2 + 1))
                                s_ps = ps_s.tile([P, CH], f32, tag="s")
                                nc.tensor.matmul(
                                    s_ps[:], kq[hs, ts(j, P)],
                                    qq[hs, ts(c, CH)],
                                    start=True, stop=True)
                                pt = pt_pool.tile([P, CH], bf16, tag="pt")
                                if j >= 4 * c:  # diagonal block
                                    off = P * (j - 4 * c)
                                    if off > 0:
                                        nc.vector.memset(pt[:, 0:off], 0.0)
                                    nc.scalar.activation(
                                        pt[:, off:CH], s_ps[:, off:CH],
                                        mybir.ActivationFunctionType.Exp,
                                        bias=neg12[:], scale=1.0)
                                    nc.vector.tensor_mul(
                                        pt[:, off:off + P],
                                        pt[:, off:off + P], m01_blk[:])
                                else:
                                    nc.scalar.activation(
                                        pt[:], s_ps[:],
                                        mybir.ActivationFunctionType.Exp,
                                        bias=neg12[:], scale=1.0)
                                nc.tensor.matmul(
                                    attn_ps[DK * h2:DK * (h2 + 1), :],
                                    v[j][:, ts(2 * p + h2, DK)], pt[:],
                                    start=st, stop=sp, skip_group_check=True)
                                nc.tensor.matmul(
                                    rs_ps[32 * h2:32 * h2 + 1, :],
                                    ones_col[:], pt[:],
                                    start=st, stop=sp, skip_group_check=True)
                        rs_sb = rs_pool.tile([33, CH], f32, tag="rs_sb")
                        nc.vector.reciprocal(rs_sb[:], rs_ps[:])
                        scrA = scr_pool.tile([1, CH], f32, tag="scrA")
                        scrB = scr_pool.tile([1, CH], f32, tag="scrB")
                        nc.sync.dma_start(scrA[:], rs_sb[0:1, :])
                        nc.sync.dma_start(scrB[:], rs_sb[32:33, :])
                        bcast = bc_pool.tile([P, CH], f32, tag="bc")
                        nc.sync.dma_start(
                            bcast[0:DK, :],
                            scrA[0:1, :].to_broadcast((DK, CH)))
                        nc.sync.dma_start(
                            bcast[DK:P, :],
                            scrB[0:1, :].to_broadcast((DK, CH)))
                        nc.vector.tensor_mul(
                            at[p][:, ts(c, CH)], attn_ps[:], bcast[:])

                    # out-projection for this chunk's token tiles (shares
                    # the ps_at pool slots)
                    for t in range(4 * c, 4 * c + 4):
                        for dc in range(2):
                            acc = ps_at.tile([P, 512], f32, tag="at",
                                             name="oacc")
                            for kk in range(4):
                                nc.tensor.matmul(
                                    acc[:], at[kk][:, ts(t, P)],
                                    wout_sb[:, kk * 1024 + dc * 512:
                                            kk * 1024 + dc * 512 + 512],
                                    start=(kk == 0), stop=(kk == 3))
                            o_sb = osb_pool.tile([P, 512], f32, tag="o_sb")
                            nc.vector.tensor_copy(o_sb[:], acc[:])
                            nc.sync.dma_start(
                                out_d[ts(t, P), ts(dc, 512)], o_sb[:])

    nc.compile()
    return nc


def _get_program():
    if "nc" not in _CACHE:
        _CACHE["nc"] = _build_program()
    return _CACHE["nc"]


def _prep_core_inputs(x, attn_mask, Wqkv, bqkv, Wout):
    """Per-core host-side sharding + DMA-friendly layouts."""
    # partial diagonal block: m01[ki_rel, qi_rel] = 1 iff qi_rel >= ki_rel
    m01 = np.triu(np.ones((P, P), np.float32)).astype(BF16)

    in_maps = []
    for core in range(NCORES):
        b, g = core // 2, core % 2
        xt = np.ascontiguousarray(
            x[b].T.reshape(KT, P, T).transpose(1, 0, 2).reshape(P, KT * T)
        ).astype(BF16)
        wq = Wqkv[:, 512 * g:512 * g + 512] * np.float32(0.125)
        wk = Wqkv[:, 1024 + 512 * g:1024 + 512 * g + 512]
        wqk = np.concatenate([wq, wk], axis=1)  # [1024, 1024]
        wqk = np.ascontiguousarray(
            wqk.reshape(KT, P, 8, P).transpose(1, 2, 0, 3).reshape(P, 8192)
        ).astype(BF16)
        wv = Wqkv[:, 2048 + 512 * g:2048 + 512 * g + 512]
        wv = np.ascontiguousarray(
            wv.reshape(KT, P, 512).transpose(1, 0, 2).reshape(P, KT * 512)
        ).astype(BF16)
        wo = Wout[512 * g:512 * g + 512, :]
        wo = np.ascontiguousarray(
            wo.reshape(4, P, 1024).transpose(1, 0, 2).reshape(P, 4096)
        ).astype(BF16)
        bq = bqkv[512 * g:512 * g + 512] * np.float32(0.125)
        bk = bqkv[1024 + 512 * g:1024 + 512 * g + 512]
        bqk = np.ascontiguousarray(
            np.concatenate([bq, bk]).reshape(8, P).T)
        bv = np.ascontiguousarray(
            bqkv[2048 + 512 * g:2048 + 512 * g + 512].reshape(1, 512)
        ).astype(BF16)
        in_maps.append({"xt": xt, "wqk": wqk, "wv": wv, "wout": wo,
                        "m01": m01, "bqk": bqk, "bv": bv})
    return in_maps


def _mask_is_causal(attn_mask):
    zero = (attn_mask == 0.0)
    if not np.array_equal(zero, np.tril(np.ones((T, T), dtype=bool))):
        return False
    return bool(np.all(attn_mask[~zero] <= np.float32(-50.0)))


def _numpy_fallback(x, attn_mask, Wqkv, bqkv, Wout, bout):
    qkv = x @ Wqkv + bqkv
    qkv = qkv.reshape(B, T, 3, H, DK).transpose(2, 0, 3, 1, 4)
    q, k, vv = qkv[0], qkv[1], qkv[2]
    scores = np.einsum("bhqd,bhkd->bhqk", q, k) / np.float32(np.sqrt(DK))
    scores = scores + attn_mask
    scores -= scores.max(axis=-1, keepdims=True)
    e = np.exp(scores)
    probs = e / e.sum(axis=-1, keepdims=True)
    attn = np.einsum("bhqk,bhkd->bhqd", probs, vv)
    attn = attn.transpose(0, 2, 1, 3).reshape(B, T, D)
    return (attn @ Wout + bout).astype(np.float32)


def _run(inputs, trace=False):
    from concourse.bass_utils import run_bass_kernel_spmd

    x = np.asarray(inputs["x"], dtype=np.float32)
    attn_mask = np.asarray(inputs["attn_mask"], dtype=np.float32)
    Wqkv = np.asarray(inputs["Wqkv"], dtype=np.float32)
    bqkv = np.asarray(inputs["bqkv"], dtype=np.float32)
    Wout = np.asarray(inputs["Wout"], dtype=np.float32)
    bout = np.asarray(inputs["bout"], dtype=np.float32)

    if not _mask_is_causal(attn_mask):
        return _numpy_fallback(x, attn_mask, Wqkv, bqkv, Wout, bout), None

    nc = _get_program()
    in_maps = _prep_core_inputs(x, attn_mask, Wqkv, bqkv, Wout)
    res = run_bass_kernel_spmd(nc, in_maps, list(range(NCORES)), trace=trace)
    out = np.empty((B, T, D), np.float32)
    for b in range(B):
        out[b] = res.results[2 * b]["out"] + res.results[2 * b + 1]["out"] + bout
    return out, res.exec_time_ns


def kernel(**inputs) -> np.ndarray:
    out, _ = _run(inputs, trace=False)
    return out


# revision 12
# speedup vs baseline: 1.3269x; 1.3269x over previous
"""Masked multi-head self-attention on 8 Trainium2 NeuronCores.

Sharding: core c handles batch b = c // 2 and head-group g = c % 2
(8 of 16 heads).  Data-parallel over B, tensor-parallel over heads for
qkv_proj (column split) / out_proj (row split).  The [T,T] causal mask
is exploited structurally (tile skipping); the host verifies the mask
is causal and falls back to numpy otherwise.  Host sums the two
head-group partial outputs per batch and adds bout.
"""

import numpy as np
import ml_dtypes

BF16 = ml_dtypes.bfloat16

B = 4
T = 2048
D = 1024
H = 16
DK = 64
P = 128
NCORES = 8

KT = D // P            # 8   k-tiles over d_model
TTILES = T // P        # 16  tiles over tokens
NCH = 4                # qi chunks of 512
CH = T // NCH          # 512
KITILES = T // P       # 16  ki tiles

_CACHE = {}


def _build_program():
    import concourse.bass as bass
    import concourse.tile as tile
    from concourse import bacc, mybir
    from contextlib import ExitStack

    f32 = mybir.dt.float32
    bf16 = mybir.dt.bfloat16
    nc = bacc.Bacc("TRN2", target_bir_lowering=False, debug=False,
                   num_devices=NCORES)

    xt_d = nc.declare_dram_parameter("xt", [P, KT * T], bf16, isOutput=False)
    wqk_d = nc.declare_dram_parameter("wqk", [P, 8 * 1024], bf16, isOutput=False)
    wv_d = nc.declare_dram_parameter("wv", [P, KT * 512], bf16, isOutput=False)
    wout_d = nc.declare_dram_parameter("wout", [P, 4 * 1024], bf16, isOutput=False)
    m01_d = nc.declare_dram_parameter("m01", [P, P], bf16, isOutput=False)
    bqk_d = nc.declare_dram_parameter("bqk", [P, 8], f32, isOutput=False)
    bv_d = nc.declare_dram_parameter("bv", [1, 512], bf16, isOutput=False)
    out_d = nc.declare_dram_parameter("out", [T, D], f32, isOutput=True)

    ts = bass.ts

    with tile.TileContext(nc) as tc, ExitStack() as top:
        const = top.enter_context(tc.tile_pool(name="const", bufs=1))
        qk_pool = top.enter_context(tc.tile_pool(name="qk", bufs=1))
        v_pool = top.enter_context(tc.tile_pool(name="v", bufs=1))

        ones_col = const.tile([P, 1], bf16, tag="ones_col")
        ones_row = const.tile([1, P], bf16, tag="ones_row")
        neg12 = const.tile([P, 1], f32, tag="neg12")
        bqk_sb = const.tile([P, 8], f32, tag="bqk")
        bv_sb = const.tile([1, 512], bf16, tag="bv")
        nc.vector.memset(ones_col[:], 1.0)
        nc.vector.memset(ones_row[:], 1.0)
        nc.vector.memset(neg12[:], -12.0)
        nc.sync.dma_start(bqk_sb[:], bqk_d[:])
        nc.sync.dma_start(bv_sb[:], bv_d[:])

        # Persistent intermediates: qkT [1024, T] as 8 tiles (i<4: q of head
        # pair i, scaled by 1/8 on host; i>=4: k of pair i-4); v [T, 512] as
        # 16 tiles; attnT [512, T] as 4 tiles.
        qk = [qk_pool.tile([P, T], bf16, tag=f"qk{i}", name=f"qk{i}")
              for i in range(8)]
        v = [v_pool.tile([P, 512], bf16, tag=f"v{t}", name=f"v{t}")
             for t in range(TTILES)]

        # ---- Phase 1: QKV projection -------------------------------------
        with ExitStack() as ph1:
            xt_pool = ph1.enter_context(tc.tile_pool(name="xt", bufs=1))
            wqk_pool = ph1.enter_context(tc.tile_pool(name="wqk", bufs=2))
            wv_pool = ph1.enter_context(tc.tile_pool(name="wv", bufs=1))
            ps1 = ph1.enter_context(tc.tile_pool(name="ps1", bufs=3, space="PSUM"))

            xt = []
            for kt in range(KT):
                xti = xt_pool.tile([P, T], bf16, tag=f"xt{kt}", name=f"xt{kt}")
                nc.sync.dma_start(xti[:], xt_d[:, ts(kt, T)])
                xt.append(xti)
            wv_sb = wv_pool.tile([P, KT * 512], bf16, tag="wv")
            nc.sync.dma_start(wv_sb[:], wv_d[:])

            # qkT[i] = (Wqk col-tile i).T @ x.T   -> [128 dcol, T]
            for i in range(8):
                wqk_sb = wqk_pool.tile([P, 1024], bf16, tag="wqk")
                nc.sync.dma_start(wqk_sb[:], wqk_d[:, ts(i, 1024)])
                for n in range(NCH):
                    acc = ps1.tile([P, CH], f32, tag="ps1")
                    for kt in range(KT):
                        nc.tensor.matmul(
                            acc[:], wqk_sb[:, ts(kt, P)], xt[kt][:, ts(n, CH)],
                            start=(kt == 0), stop=(kt == KT - 1))
                    nc.vector.tensor_scalar_add(
                        qk[i][:, ts(n, CH)], acc[:], bqk_sb[:, i:i + 1])

            # v[t] = x-tile.T @ Wv + bv           -> [128 tok, 512 dcol]
            for t in range(TTILES):
                acc = ps1.tile([P, 512], f32, tag="ps1")
                for kt in range(KT):
                    nc.tensor.matmul(
                        acc[:], xt[kt][:, ts(t, P)], wv_sb[:, ts(kt, 512)],
                        start=(kt == 0), stop=False)
                nc.tensor.matmul(acc[:], ones_row[:], bv_sb[:],
                                 start=False, stop=True)
                nc.vector.tensor_copy(v[t][:], acc[:])

        # ---- Phase 2: attention ------------------------------------------
        with ExitStack() as ph2:
            at_pool = ph2.enter_context(tc.tile_pool(name="at", bufs=1))
            wout_pool = ph2.enter_context(tc.tile_pool(name="wout", bufs=1))
            m01_pool = ph2.enter_context(tc.tile_pool(name="m01", bufs=1))
            pt_pool = ph2.enter_context(tc.tile_pool(name="pt", bufs=6))
            bc_pool = ph2.enter_context(tc.tile_pool(name="bc", bufs=2))
            scr_pool = ph2.enter_context(
                tc.tile_pool(name="scr", bufs=2, space="DRAM"))
            rs_pool = ph2.enter_context(tc.tile_pool(name="rs", bufs=2))
            at = [at_pool.tile([P, T], bf16, tag=f"at{p}", name=f"at{p}")
                  for p in range(4)]
            wout_sb = wout_pool.tile([P, 4 * 1024], bf16, tag="wout")
            nc.sync.dma_start(wout_sb[:], wout_d[:])
            m01_blk = m01_pool.tile([P, P], bf16, tag="m01")
            nc.sync.dma_start(m01_blk[:], m01_d[:])

            osb_pool = ph2.enter_context(tc.tile_pool(name="osb", bufs=4))
            with ExitStack() as ph2p:
                ps_s = ph2p.enter_context(
                    tc.tile_pool(name="ps_s", bufs=2, space="PSUM"))
                ps_at = ph2p.enter_context(
                    tc.tile_pool(name="ps_at", bufs=2, space="PSUM"))
                ps_rs = ph2p.enter_context(
                    tc.tile_pool(name="ps_rs", bufs=2, space="PSUM"))

                for c in range(NCH):
                    nki = 4 * (c + 1)  # causal: ki tiles 0..nki-1
                    for p in range(4):
                        kq = qk[4 + p]  # kT tiles for this pair
                        qq = qk[p]      # qT tiles
                        attn_ps = ps_at.tile([P, CH], f32, tag="at")
                        rs_ps = ps_rs.tile([33, CH], f32, tag="rs")
                        nc.vector.memset(rs_ps[0:32, :], 1.0)
                        for j in range(nki):
                            st = (j == 0)
                            sp = (j == nki - 1)
                            s_ps = ps_s.tile([P, 1024], f32, tag="s")
                            nc.tensor.matmul(
                                s_ps[:, 0:512],
                                kq[0:DK, ts(j, P)], qq[0:DK, ts(c, CH)],
                                start=True, stop=True)
                            nc.tensor.matmul(
                                s_ps[:, 512:1024],
                                kq[DK:P, ts(j, P)], qq[DK:P, ts(c, CH)],
                                start=True, stop=True)
                            pt = pt_pool.tile([P, 1024], bf16, tag="pt")
                            if j >= 4 * c:  # diagonal block
                                off = P * (j - 4 * c)
                                for base in (0, 512):
                                    if off > 0:
                                        nc.vector.memset(
                                            pt[:, base:base + off], 0.0)
                                    nc.scalar.activation(
                                        pt[:, base + off:base + 512],
                                        s_ps[:, base + off:base + 512],
                                        mybir.ActivationFunctionType.Exp,
                                        bias=neg12[:], scale=1.0)
                                    nc.vector.tensor_mul(
                                        pt[:, base + off:base + off + P],
                                        pt[:, base + off:base + off + P],
                                        m01_blk[:])
                            else:
                                nc.scalar.activation(
                                    pt[:], s_ps[:],
                                    mybir.ActivationFunctionType.Exp,
                                    bias=neg12[:], scale=1.0)
                            nc.tensor.matmul(
                                attn_ps[0:DK, :],
                                v[j][:, ts(2 * p, DK)], pt[:, 0:512],
                                start=st, stop=sp, skip_group_check=True)
                            nc.tensor.matmul(
                                attn_ps[DK:P, :],
                                v[j][:, ts(2 * p + 1, DK)], pt[:, 512:1024],
                                start=st, stop=sp, skip_group_check=True)
                            nc.tensor.matmul(
                                rs_ps[0:1, :], ones_col[:], pt[:, 0:512],
                                start=st, stop=sp, skip_group_check=True)
                            nc.tensor.matmul(
                                rs_ps[32:33, :], ones_col[:], pt[:, 512:1024],
                                start=st, stop=sp, skip_group_check=True)
                        rs_sb = rs_pool.tile([33, CH], f32, tag="rs_sb")
                        nc.vector.reciprocal(rs_sb[:], rs_ps[:])
                        scrA = scr_pool.tile([1, CH], f32, tag="scrA")
                        scrB = scr_pool.tile([1, CH], f32, tag="scrB")
                        nc.sync.dma_start(scrA[:], rs_sb[0:1, :])
                        nc.sync.dma_start(scrB[:], rs_sb[32:33, :])
                        bcast = bc_pool.tile([P, CH], f32, tag="bc")
                        nc.sync.dma_start(
                            bcast[0:DK, :],
                            scrA[0:1, :].to_broadcast((DK, CH)))
                        nc.sync.dma_start(
                            bcast[DK:P, :],
                            scrB[0:1, :].to_broadcast((DK, CH)))
                        nc.vector.tensor_mul(
                            at[p][:, ts(c, CH)], attn_ps[:], bcast[:])

            # ---- Phase 3: output projection ------------------------------
            with ExitStack() as ph3:
                ps_o = ph3.enter_context(
                    tc.tile_pool(name="ps_o", bufs=4, space="PSUM"))
                for t in range(TTILES):
                    for dc in range(2):
                        acc = ps_o.tile([P, 512], f32, tag="o")
                        for kk in range(4):
                            nc.tensor.matmul(
                                acc[:], at[kk][:, ts(t, P)],
                                wout_sb[:, kk * 1024 + dc * 512:
                                        kk * 1024 + dc * 512 + 512],
                                start=(kk == 0), stop=(kk == 3))
                        o_sb = osb_pool.tile([P, 512], f32, tag="o_sb")
                        nc.vector.tensor_copy(o_sb[:], acc[:])
                        nc.sync.dma_start(
                            out_d[ts(t, P), ts(dc, 512)], o_sb[:])

    nc.compile()
    return nc


def _get_program():
    if "nc" not in _CACHE:
        _CACHE["nc"] = _build_program()
    return _CACHE["nc"]


def _prep_core_inputs(x, attn_mask, Wqkv, bqkv, Wout):
    """Per-core host-side sharding + DMA-friendly layouts."""
    # partial diagonal block: m01[ki_rel, qi_rel] = 1 iff qi_rel >= ki_rel
    m01 = np.triu(np.ones((P, P), np.float32)).astype(BF16)

    in_maps = []
    for core in range(NCORES):
        b, g = core // 2, core % 2
        xt = np.ascontiguousarray(
            x[b].T.reshape(KT, P, T).transpose(1, 0, 2).reshape(P, KT * T)
        ).astype(BF16)
        wq = Wqkv[:, 512 * g:512 * g + 512] * np.float32(0.125)
        wk = Wqkv[:, 1024 + 512 * g:1024 + 512 * g + 512]
        wqk = np.concatenate([wq, wk], axis=1)  # [1024, 1024]
        wqk = np.ascontiguousarray(
            wqk.reshape(KT, P, 8, P).transpose(1, 2, 0, 3).reshape(P, 8192)
        ).astype(BF16)
        wv = Wqkv[:, 2048 + 512 * g:2048 + 512 * g + 512]
        wv = np.ascontiguousarray(
            wv.reshape(KT, P, 512).transpose(1, 0, 2).reshape(P, KT * 512)
        ).astype(BF16)
        wo = Wout[512 * g:512 * g + 512, :]
        wo = np.ascontiguousarray(
            wo.reshape(4, P, 1024).transpose(1, 0, 2).reshape(P, 4096)
        ).astype(BF16)
        bq = bqkv[512 * g:512 * g + 512] * np.float32(0.125)
        bk = bqkv[1024 + 512 * g:1024 + 512 * g + 512]
        bqk = np.ascontiguousarray(
            np.concatenate([bq, bk]).reshape(8, P).T)
        bv = np.ascontiguousarray(
            bqkv[2048 + 512 * g:2048 + 512 * g + 512].reshape(1, 512)
        ).astype(BF16)
        in_maps.append({"xt": xt, "wqk": wqk, "wv": wv, "wout": wo,
                        "m01": m01, "bqk": bqk, "bv": bv})
    return in_maps


def _mask_is_causal(attn_mask):
    zero = (attn_mask == 0.0)
    if not np.array_equal(zero, np.tril(np.ones((T, T), dtype=bool))):
        return False
    return bool(np.all(attn_mask[~zero] <= np.float32(-50.0)))


def _numpy_fallback(x, attn_mask, Wqkv, bqkv, Wout, bout):
    qkv = x @ Wqkv + bqkv
    qkv = qkv.reshape(B, T, 3, H, DK).transpose(2, 0, 3, 1, 4)
    q, k, vv = qkv[0], qkv[1], qkv[2]
    scores = np.einsum("bhqd,bhkd->bhqk", q, k) / np.float32(np.sqrt(DK))
    scores = scores + attn_mask
    scores -= scores.max(axis=-1, keepdims=True)
    e = np.exp(scores)
    probs = e / e.sum(axis=-1, keepdims=True)
    attn = np.einsum("bhqk,bhkd->bhqd", probs, vv)
    attn = attn.transpose(0, 2, 1, 3).reshape(B, T, D)
    return (attn @ Wout + bout).astype(np.float32)


def _run(inputs, trace=False):
    from concourse.bass_utils import run_bass_kernel_spmd

    x = np.asarray(inputs["x"], dtype=np.float32)
    attn_mask = np.asarray(inputs["attn_mask"], dtype=np.float32)
    Wqkv = np.asarray(inputs["Wqkv"], dtype=np.float32)
    bqkv = np.asarray(inputs["bqkv"], dtype=np.float32)
    Wout = np.asarray(inputs["Wout"], dtype=np.float32)
    bout = np.asarray(inputs["bout"], dtype=np.float32)

    if not _mask_is_causal(attn_mask):
        return _numpy_fallback(x, attn_mask, Wqkv, bqkv, Wout, bout), None

    nc = _get_program()
    in_maps = _prep_core_inputs(x, attn_mask, Wqkv, bqkv, Wout)
    res = run_bass_kernel_spmd(nc, in_maps, list(range(NCORES)), trace=trace)
    out = np.empty((B, T, D), np.float32)
    for b in range(B):
        out[b] = res.results[2 * b]["out"] + res.results[2 * b + 1]["out"] + bout
    return out, res.exec_time_ns


def kernel(**inputs) -> np.ndarray:
    out, _ = _run(inputs, trace=False)
    return out


# revision 15
# speedup vs baseline: 1.4570x; 1.0980x over previous
"""Masked multi-head self-attention on 8 Trainium2 NeuronCores.

Sharding: core c handles batch b = c // 2 and head-group g = c % 2
(8 of 16 heads).  Data-parallel over B, tensor-parallel over heads for
qkv_proj (column split) / out_proj (row split).  The [T,T] causal mask
is exploited structurally (tile skipping); the host verifies the mask
is causal and falls back to numpy otherwise.  Host sums the two
head-group partial outputs per batch and adds bout.
"""

import numpy as np
import ml_dtypes

BF16 = ml_dtypes.bfloat16

B = 4
T = 2048
D = 1024
H = 16
DK = 64
P = 128
NCORES = 8

KT = D // P            # 8   k-tiles over d_model
TTILES = T // P        # 16  tiles over tokens
NCH = 4                # qi chunks of 512
CH = T // NCH          # 512
KITILES = T // P       # 16  ki tiles

_CACHE = {}


def _build_program():
    import concourse.bass as bass
    import concourse.tile as tile
    from concourse import bacc, mybir
    from contextlib import ExitStack

    f32 = mybir.dt.float32
    bf16 = mybir.dt.bfloat16
    nc = bacc.Bacc("TRN2", target_bir_lowering=False, debug=False,
                   num_devices=NCORES)

    xt_d = nc.declare_dram_parameter("xt", [P, KT * T], bf16, isOutput=False)
    wqk_d = nc.declare_dram_parameter("wqk", [P, 8 * 1024], bf16, isOutput=False)
    wv_d = nc.declare_dram_parameter("wv", [P, KT * 512], bf16, isOutput=False)
    wout_d = nc.declare_dram_parameter("wout", [P, 4 * 1024], bf16, isOutput=False)
    m01_d = nc.declare_dram_parameter("m01", [P, P], bf16, isOutput=False)
    bqk_d = nc.declare_dram_parameter("bqk", [P, 8], f32, isOutput=False)
    bv_d = nc.declare_dram_parameter("bv", [1, 512], bf16, isOutput=False)
    out_d = nc.declare_dram_parameter("out", [T, D], f32, isOutput=True)

    ts = bass.ts

    with tile.TileContext(nc) as tc, ExitStack() as top:
        const = top.enter_context(tc.tile_pool(name="const", bufs=1))
        qk_pool = top.enter_context(tc.tile_pool(name="qk", bufs=1))
        v_pool = top.enter_context(tc.tile_pool(name="v", bufs=1))

        ones_col = const.tile([P, 1], bf16, tag="ones_col")
        ones_row = const.tile([1, P], bf16, tag="ones_row")
        neg12 = const.tile([P, 1], f32, tag="neg12")
        bqk_sb = const.tile([P, 8], f32, tag="bqk")
        bv_sb = const.tile([1, 512], bf16, tag="bv")
        nc.vector.memset(ones_col[:], 1.0)
        nc.vector.memset(ones_row[:], 1.0)
        nc.vector.memset(neg12[:], -12.0)
        nc.sync.dma_start(bqk_sb[:], bqk_d[:])
        nc.sync.dma_start(bv_sb[:], bv_d[:])

        # Persistent intermediates: qkT [1024, T] as 8 tiles (i<4: q of head
        # pair i, scaled by 1/8 on host; i>=4: k of pair i-4); v [T, 512] as
        # 16 tiles; attnT [512, T] as 4 tiles.
        qk = [qk_pool.tile([P, T], bf16, tag=f"qk{i}", name=f"qk{i}")
              for i in range(8)]
        v = [v_pool.tile([P, 512], bf16, tag=f"v{t}", name=f"v{t}")
             for t in range(TTILES)]

        # ---- Phase 1: QKV projection -------------------------------------
        with ExitStack() as ph1:
            xt_pool = ph1.enter_context(tc.tile_pool(name="xt", bufs=1))
            wqk_pool = ph1.enter_context(tc.tile_pool(name="wqk", bufs=2))
            wv_pool = ph1.enter_context(tc.tile_pool(name="wv", bufs=1))
            ps1 = ph1.enter_context(tc.tile_pool(name="ps1", bufs=3, space="PSUM"))

            xt = []
            for kt in range(KT):
                xti = xt_pool.tile([P, T], bf16, tag=f"xt{kt}", name=f"xt{kt}")
                nc.sync.dma_start(xti[:], xt_d[:, ts(kt, T)])
                xt.append(xti)
            wv_sb = wv_pool.tile([P, KT * 512], bf16, tag="wv")
            nc.sync.dma_start(wv_sb[:], wv_d[:])

            # qkT[i] = (Wqk col-tile i).T @ x.T   -> [128 dcol, T]
            for i in range(8):
                wqk_sb = wqk_pool.tile([P, 1024], bf16, tag="wqk")
                nc.sync.dma_start(wqk_sb[:], wqk_d[:, ts(i, 1024)])
                for n in range(NCH):
                    acc = ps1.tile([P, CH], f32, tag="ps1")
                    for kt in range(KT):
                        nc.tensor.matmul(
                            acc[:], wqk_sb[:, ts(kt, P)], xt[kt][:, ts(n, CH)],
                            start=(kt == 0), stop=(kt == KT - 1))
                    nc.vector.tensor_scalar_add(
                        qk[i][:, ts(n, CH)], acc[:], bqk_sb[:, i:i + 1])

            # v[t] = x-tile.T @ Wv + bv           -> [128 tok, 512 dcol]
            for t in range(TTILES):
                acc = ps1.tile([P, 512], f32, tag="ps1")
                for kt in range(KT):
                    nc.tensor.matmul(
                        acc[:], xt[kt][:, ts(t, P)], wv_sb[:, ts(kt, 512)],
                        start=(kt == 0), stop=False)
                nc.tensor.matmul(acc[:], ones_row[:], bv_sb[:],
                                 start=False, stop=True)
                nc.vector.tensor_copy(v[t][:], acc[:])

        # ---- Phase 2: attention ------------------------------------------
        with ExitStack() as ph2:
            at_pool = ph2.enter_context(tc.tile_pool(name="at", bufs=1))
            wout_pool = ph2.enter_context(tc.tile_pool(name="wout", bufs=1))
            m01_pool = ph2.enter_context(tc.tile_pool(name="m01", bufs=1))
            pt_pool = ph2.enter_context(tc.tile_pool(name="pt", bufs=6))
            bc_pool = ph2.enter_context(tc.tile_pool(name="bc", bufs=2))
            scr_pool = ph2.enter_context(
                tc.tile_pool(name="scr", bufs=2, space="DRAM"))
            rs_pool = ph2.enter_context(tc.tile_pool(name="rs", bufs=2))
            at = [at_pool.tile([P, T], bf16, tag=f"at{p}", name=f"at{p}")
                  for p in range(4)]
            wout_sb = wout_pool.tile([P, 4 * 1024], bf16, tag="wout")
            nc.sync.dma_start(wout_sb[:], wout_d[:])
            m01_blk = m01_pool.tile([P, P], bf16, tag="m01")
            nc.sync.dma_start(m01_blk[:], m01_d[:])

            osb_pool = ph2.enter_context(tc.tile_pool(name="osb", bufs=4))
            with ExitStack() as ph2p:
                ps_s = ph2p.enter_context(
                    tc.tile_pool(name="ps_s", bufs=2, space="PSUM"))
                ps_at = ph2p.enter_context(
                    tc.tile_pool(name="ps_at", bufs=2, space="PSUM"))
                ps_rs = ph2p.enter_context(
                    tc.tile_pool(name="ps_rs", bufs=2, space="PSUM"))

                for c in range(NCH):
                    nki = 4 * (c + 1)  # causal: ki tiles 0..nki-1
                    for p in range(4):
                        kq = qk[4 + p]  # kT tiles for this pair
                        qq = qk[p]      # qT tiles
                        attn_ps = ps_at.tile([P, CH], f32, tag="at")
                        rs_ps = ps_rs.tile([P, CH], f32, tag="rs")
                        nc.vector.memset(rs_ps[0:32, :], 1.0)
                        for j in range(nki):
                            st = (j == 0)
                            sp = (j == nki - 1)
                            s_ps = ps_s.tile([P, 1024], f32, tag="s")
                            nc.tensor.matmul(
                                s_ps[:, 0:512],
                                kq[0:DK, ts(j, P)], qq[0:DK, ts(c, CH)],
                                start=True, stop=True)
                            nc.tensor.matmul(
                                s_ps[:, 512:1024],
                                kq[DK:P, ts(j, P)], qq[DK:P, ts(c, CH)],
                                start=True, stop=True)
                            pt = pt_pool.tile([P, 1024], bf16, tag="pt")
                            if j >= 4 * c:  # diagonal block
                                off = P * (j - 4 * c)
                                for base in (0, 512):
                                    if off > 0:
                                        nc.vector.memset(
                                            pt[:, base:base + off], 0.0)
                                    nc.scalar.activation(
                                        pt[:, base + off:base + 512],
                                        s_ps[:, base + off:base + 512],
                                        mybir.ActivationFunctionType.Exp,
                                        bias=neg12[:], scale=1.0)
                                    nc.vector.tensor_mul(
                                        pt[:, base + off:base + off + P],
                                        pt[:, base + off:base + off + P],
                                        m01_blk[:])
                            else:
                                nc.scalar.activation(
                                    pt[:], s_ps[:],
                                    mybir.ActivationFunctionType.Exp,
                                    bias=neg12[:], scale=1.0)
                            nc.tensor.matmul(
                                attn_ps[0:DK, :],
                                v[j][:, ts(2 * p, DK)], pt[:, 0:512],
                                start=st, stop=sp, skip_group_check=True)
                            nc.tensor.matmul(
                                attn_ps[DK:P, :],
                                v[j][:, ts(2 * p + 1, DK)], pt[:, 512:1024],
                                start=st, stop=sp, skip_group_check=True)
                            nc.tensor.matmul(
                                rs_ps[0:1, :], ones_col[:], pt[:, 0:512],
                                start=st, stop=sp, skip_group_check=True)
                            nc.tensor.matmul(
                                rs_ps[32:33, :], ones_col[:], pt[:, 512:1024],
                                start=st, stop=sp, skip_group_check=True)
                        rs_sb = rs_pool.tile([33, CH], f32, tag="rs_sb")
                        nc.vector.reciprocal_approx_fast(
                            rs_sb[:], rs_ps[0:33, :])
                        scrA = scr_pool.tile([1, CH], f32, tag="scrA")
                        scrB = scr_pool.tile([1, CH], f32, tag="scrB")
                        nc.sync.dma_start(scrA[:], rs_sb[0:1, :])
                        nc.sync.dma_start(scrB[:], rs_sb[32:33, :])
                        bcast = bc_pool.tile([P, CH], f32, tag="bc")
                        nc.sync.dma_start(
                            bcast[0:DK, :],
                            scrA[0:1, :].to_broadcast((DK, CH)))
                        nc.sync.dma_start(
                            bcast[DK:P, :],
                            scrB[0:1, :].to_broadcast((DK, CH)))
                        nc.vector.tensor_mul(
                            at[p][:, ts(c, CH)], attn_ps[:], bcast[:])

                    # out-projection for this chunk's token tiles
                    # (reuses the ps_rs slots, which are short-lived)
                    for t in range(4 * c, 4 * c + 4):
                        for dc in range(2):
                            acc = ps_rs.tile([P, 512], f32, tag="rs",
                                             name="oacc")
                            for kk in range(4):
                                nc.tensor.matmul(
                                    acc[:], at[kk][:, ts(t, P)],
                                    wout_sb[:, kk * 1024 + dc * 512:
                                            kk * 1024 + dc * 512 + 512],
                                    start=(kk == 0), stop=(kk == 3))
                            o_sb = osb_pool.tile([P, 512], f32, tag="o_sb")
                            nc.vector.tensor_copy(o_sb[:], acc[:])
                            nc.sync.dma_start(
                                out_d[ts(t, P), ts(dc, 512)], o_sb[:])

    nc.compile()
    return nc


def _get_program():
    if "nc" not in _CACHE:
        _CACHE["nc"] = _build_program()
    return _CACHE["nc"]


def _prep_core_inputs(x, attn_mask, Wqkv, bqkv, Wout):
    """Per-core host-side sharding + DMA-friendly layouts."""
    # partial diagonal block: m01[ki_rel, qi_rel] = 1 iff qi_rel >= ki_rel
    m01 = np.triu(np.ones((P, P), np.float32)).astype(BF16)

    in_maps = []
    for core in range(NCORES):
        b, g = core // 2, core % 2
        xt = np.ascontiguousarray(
            x[b].T.reshape(KT, P, T).transpose(1, 0, 2).reshape(P, KT * T)
        ).astype(BF16)
        wq = Wqkv[:, 512 * g:512 * g + 512] * np.float32(0.125)
        wk = Wqkv[:, 1024 + 512 * g:1024 + 512 * g + 512]
        wqk = np.concatenate([wq, wk], axis=1)  # [1024, 1024]
        wqk = np.ascontiguousarray(
            wqk.reshape(KT, P, 8, P).transpose(1, 2, 0, 3).reshape(P, 8192)
        ).astype(BF16)
        wv = Wqkv[:, 2048 + 512 * g:2048 + 512 * g + 512]
        wv = np.ascontiguousarray(
            wv.reshape(KT, P, 512).transpose(1, 0, 2).reshape(P, KT * 512)
        ).astype(BF16)
        wo = Wout[512 * g:512 * g + 512, :]
        wo = np.ascontiguousarray(
            wo.reshape(4, P, 1024).transpose(1, 0, 2).reshape(P, 4096)
        ).astype(BF16)
        bq = bqkv[512 * g:512 * g + 512] * np.float32(0.125)
        bk = bqkv[1024 + 512 * g:1024 + 512 * g + 512]
        bqk = np.ascontiguousarray(
            np.concatenate([bq, bk]).reshape(8, P).T)
        bv = np.ascontiguousarray(
            bqkv[2048 + 512 * g:2048 + 512 * g + 512].reshape(1, 512)
        ).astype(BF16)
        in_maps.append({"xt": xt, "wqk": wqk, "wv": wv, "wout": wo,
                        "m01": m01, "bqk": bqk, "bv": bv})
    return in_maps


def _mask_is_causal(attn_mask):
    zero = (attn_mask == 0.0)
    if not np.array_equal(zero, np.tril(np.ones((T, T), dtype=bool))):
        return False
    return bool(np.all(attn_mask[~zero] <= np.float32(-50.0)))


def _numpy_fallback(x, attn_mask, Wqkv, bqkv, Wout, bout):
    qkv = x @ Wqkv + bqkv
    qkv = qkv.reshape(B, T, 3, H, DK).transpose(2, 0, 3, 1, 4)
    q, k, vv = qkv[0], qkv[1], qkv[2]
    scores = np.einsum("bhqd,bhkd->bhqk", q, k) / np.float32(np.sqrt(DK))
    scores = scores + attn_mask
    scores -= scores.max(axis=-1, keepdims=True)
    e = np.exp(scores)
    probs = e / e.sum(axis=-1, keepdims=True)
    attn = np.einsum("bhqk,bhkd->bhqd", probs, vv)
    attn = attn.transpose(0, 2, 1, 3).reshape(B, T, D)
    return (attn @ Wout + bout).astype(np.float32)


def _run(inputs, trace=False):
    from concourse.bass_utils import run_bass_kernel_spmd

    x = np.asarray(inputs["x"], dtype=np.float32)
    attn_mask = np.asarray(inputs["attn_mask"], dtype=np.float32)
    Wqkv = np.asarray(inputs["Wqkv"], dtype=np.float32)
    bqkv = np.asarray(inputs["bqkv"], dtype=np.float32)
    Wout = np.asarray(inputs["Wout"], dtype=np.float32)
    bout = np.asarray(inputs["bout"], dtype=np.float32)

    if not _mask_is_causal(attn_mask):
        return _numpy_fallback(x, attn_mask, Wqkv, bqkv, Wout, bout), None

    nc = _get_program()
    in_maps = _prep_core_inputs(x, attn_mask, Wqkv, bqkv, Wout)
    res = run_bass_kernel_spmd(nc, in_maps, list(range(NCORES)), trace=trace)
    out = np.empty((B, T, D), np.float32)
    for b in range(B):
        out[b] = res.results[2 * b]["out"] + res.results[2 * b + 1]["out"] + bout
    return out, res.exec_time_ns


def kernel(**inputs) -> np.ndarray:
    out, _ = _run(inputs, trace=False)
    return out
